# revision 34
# baseline (speedup 1.0000x reference)
"""Causal relative multi-head attention (prefill) on 8 Trainium2 NeuronCores.

Reference computation (fp32):
    q = x @ Wq.T + bq ; k = x @ Wk.T + bk ; v = x @ Wv.T + bv      [B,S,D]
    p = pos @ Wp.T + bp                                            [S,D]
    scores = causal((q+p) @ k.T / sqrt(dk)) ; attn = softmax(scores)
    out = (attn @ v) @ Wo.T + bo                                   [B,S,D]
with B=4, S=2048, D=1024, H=16, dk=64.

Sharding: batch x head-group. Core c handles batch b=c//2 and head group
g=c%2 (8 heads = 512 of the 1024 qkv/concat dims). After attention, the
pair {2b, 2b+1} exchanges bf16 attention outputs per 512-query chunk via
a small AllGather; each core then computes out-proj columns 512g:512(g+1)
over the full 1024-dim concat and writes its y half. Host stitches halves.

Key structure (v1):
  - pos projection p = pos@Wp.T + bp + bq is computed on HOST and shipped
    as pq [128,4,S] bf16; the device q-proj contracts only over x (8 steps)
    and adds pq with one DVE tensor_tensor per (dt, block).
  - 1/sqrt(dk) is folded into the exp activation's free scale operand
    (exp(0.125*s)), so no score scaling op exists anywhere.
  - The causal mask is applied on the PE: a tri[128,128] bf16 stationary
    (-240000 above the diagonal) matmul'd with an identity moving operand
    accumulates into the diagonal score block, so DVE is off the
    score->exp critical path.
  - Normalize: attn@v+rowsum accumulate into one [128,2,QB] psum pair
    (head A/B); one full-height copy to SBUF, one joint reciprocal over
    both heads' rowsum rows, two muls.
  - Tail: qb=3's B-half exchange is split per head-pair chunk (two
    1-chunk AllGathers) and phase 3 of the last chunk accumulates all
    early-available concat chunks first, so only the last small AG plus
    8 matmuls + bias are exposed at the end.
  - Startup: block-0 x and the k-weights are DMA'd in consumption-order
    chunks so the first matmul issues after ~400KB, not ~6MB.
"""

import numpy as np
import ml_dtypes

import concourse.bacc as bacc
import concourse.mybir as mybir
import concourse.tile as tile
from concourse.bass_utils import run_bass_kernel_spmd

F32 = mybir.dt.float32
BF16 = mybir.dt.bfloat16
F8 = mybir.dt.float8e4
DR = mybir.MatmulPerfMode.DoubleRow
AFT = mybir.ActivationFunctionType
ALU = mybir.AluOpType

B, S, D = 4, 2048, 1024
H, DK = 16, 64
N_CORES = 8
GROUP_DIMS = 512              # qkv dims per head group (8 heads x 64)
SB = 512                      # phase-1 seq block
NSB = S // SB                 # 4
QB = 512                      # phase-2 query block / output chunk
NQB = S // QB                 # 4
MASK_NEG = -240000.0          # pre-divided by the 0.125 exp scale

_PROG = None
_last_in_maps = None


def _build_program():
    nc = bacc.Bacc("TRN2", target_bir_lowering=False, debug=False,
                   num_devices=N_CORES)

    xpa_d = nc.dram_tensor("xpa", [NSB, 8, 128, SB], BF16, kind="ExternalInput")
    wk_d = nc.dram_tensor("wk", [4, 128, 8, 128], BF16, kind="ExternalInput")
    wv_d = nc.dram_tensor("wv", [128, 8, GROUP_DIMS], BF16, kind="ExternalInput")
    wq_d = nc.dram_tensor("wq", [128, 8, GROUP_DIMS], BF16, kind="ExternalInput")
    pq_d = nc.dram_tensor("pq", [NSB, 128, 4, SB], BF16, kind="ExternalInput")
    wo_d = nc.dram_tensor("wo", [128, 8, GROUP_DIMS], BF16, kind="ExternalInput")
    bk_d = nc.dram_tensor("bk", [128, 4], F32, kind="ExternalInput")
    bo_d = nc.dram_tensor("bo_bc", [128, GROUP_DIMS], F32, kind="ExternalInput")
    triT_d = nc.dram_tensor("triT", [128, 128], BF16, kind="ExternalInput")
    i128_d = nc.dram_tensor("i128", [128, 128], BF16, kind="ExternalInput")
    y_d = nc.dram_tensor("y", [S, GROUP_DIMS], BF16, kind="ExternalOutput")

    with tile.TileContext(nc) as tc:
        with (
            tc.tile_pool(name="wts", bufs=1) as wts,
            tc.tile_pool(name="xinA", bufs=3) as xinA,
            tc.tile_pool(name="big", bufs=1) as big,
            tc.tile_pool(name="att", bufs=3) as att,
            tc.tile_pool(name="rcp", bufs=2) as rcp,
            tc.tile_pool(name="cfp", bufs=2) as cfp,
            tc.tile_pool(name="outp", bufs=4) as outp,
            tc.tile_pool(name="cst", bufs=1) as cst,
            tc.tile_pool(name="ps", bufs=2, space="PSUM") as ps,
            tc.tile_pool(name="ps_s", bufs=2, space="PSUM") as ps_s,
            tc.tile_pool(name="ps_acc", bufs=1, space="PSUM") as ps_acc,
            tc.tile_pool(name="dram", bufs=1, space="DRAM") as dram,
        ):
            # ---- early DMAs in consumption order: k weights (dt chunks)
            # interleaved with block-0 x slices, then v/q weights + pq ----
            bk_t = cst.tile([128, 4], F32)
            nc.sync.dma_start(bk_t[:], bk_d[:])
            wk_t = wts.tile([128, 4, 8, 128], BF16)
            wv_t = wts.tile([128, 8, GROUP_DIMS], BF16)
            wq_t = wts.tile([128, 8, GROUP_DIMS], BF16)
            pq_t = wts.tile([128, 4, S], BF16)
            xp_tiles = {}

            def issue_xp(sb):
                ta = xinA.tile([128, 8, SB], BF16, tag="xpa")
                for i in range(8):
                    nc.sync.dma_start(ta[:, i, :], xpa_d[sb, i])
                nc.sync.dma_start(pq_t[:, :, sb * SB:(sb + 1) * SB], pq_d[sb])
                xp_tiles[sb] = ta

            ta0 = xinA.tile([128, 8, SB], BF16, tag="xpa")
            nc.sync.dma_start(wk_t[:, 0], wk_d[0])
            nc.sync.dma_start(ta0[:, 0, :], xpa_d[0, 0])
            nc.sync.dma_start(ta0[:, 1, :], xpa_d[0, 1])
            nc.sync.dma_start(wk_t[:, 1], wk_d[1])
            nc.sync.dma_start(ta0[:, 2, :], xpa_d[0, 2])
            nc.sync.dma_start(ta0[:, 3, :], xpa_d[0, 3])
            nc.sync.dma_start(wk_t[:, 2], wk_d[2])
            nc.sync.dma_start(ta0[:, 4, :], xpa_d[0, 4])
            nc.sync.dma_start(ta0[:, 5, :], xpa_d[0, 5])
            nc.sync.dma_start(wk_t[:, 3], wk_d[3])
            nc.sync.dma_start(ta0[:, 6, :], xpa_d[0, 6])
            nc.sync.dma_start(ta0[:, 7, :], xpa_d[0, 7])
            xp_tiles[0] = ta0
            nc.sync.dma_start(wv_t[:], wv_d[:])
            nc.sync.dma_start(pq_t[:, :, 0:SB], pq_d[0])
            nc.sync.dma_start(wq_t[:], wq_d[:])

            qT = big.tile([128, 4, S], BF16)      # q+p (unscaled), dims x seq
            kT = big.tile([128, 4, S], BF16)
            v_sb = big.tile([128, S // 128, 4, 256], BF16)  # [vA|1|vB|1]
            nc.gpsimd.memset(v_sb[:, :, :, 64:128], 1.0)
            nc.gpsimd.memset(v_sb[:, :, :, 192:256], 1.0)

            def phase1_gen(sb):
                """k -> v -> q projections for seq block sb; yields after
                small instruction bundles so it can be woven into phase 2."""
                xpa = xp_tiles.pop(sb)
                sbsl = slice(sb * SB, (sb + 1) * SB)
                for dt in range(4):
                    psk = ps.tile([128, GROUP_DIMS], F32, tag="ps")
                    for i in range(8):
                        nc.tensor.matmul(psk[:, :SB],
                                         wk_t[:, dt, i, :],
                                         xpa[:, i, :],
                                         start=(i == 0), stop=(i == 7))
                        if i % 2 == 1:
                            yield
                    nc.vector.tensor_scalar_add(
                        kT[:, dt, sbsl], psk[:, :SB], bk_t[:, dt:dt + 1])
                    yield
                for st in range(4):
                    psv = ps.tile([128, GROUP_DIMS], F32, tag="ps")
                    for i in range(8):
                        nc.tensor.matmul(psv[:],
                                         xpa[:, i, st * 128:(st + 1) * 128],
                                         wv_t[:, i, :],
                                         start=(i == 0), stop=(i == 7))
                        if i % 2 == 1:
                            yield
                    pv = psv[:].rearrange("p (a c) -> p a c", a=4)
                    t = sb * 4 + st
                    nc.vector.tensor_copy(v_sb[:, t, :, 0:64], pv[:, :, 0:64])
                    nc.vector.tensor_copy(v_sb[:, t, :, 128:192], pv[:, :, 64:128])
                    yield
                for dt in range(4):
                    psq = ps.tile([128, GROUP_DIMS], F32, tag="ps")
                    for i in range(8):
                        nc.tensor.matmul(psq[:, :SB],
                                         wq_t[:, i, dt * 128:(dt + 1) * 128],
                                         xpa[:, i, :],
                                         start=(i == 0), stop=(i == 7))
                        if i % 2 == 1:
                            yield
                    # cast PSUM fp32 -> bf16 first so the add is a clean
                    # all-bf16 tensor_tensor (2x DVE mode, no mixed dtypes)
                    qtmp = rcp.tile([128, SB], BF16, tag="qtmp")
                    nc.vector.tensor_copy(qtmp[:], psq[:, :SB])
                    yield
                    nc.vector.tensor_add(
                        qT[:, dt, sbsl], qtmp[:], pq_t[:, dt, sbsl])
                    yield

            for _ in phase1_gen(0):
                pass
            issue_xp(1)

            triT_t = cst.tile([128, 128], BF16)
            i128_t = cst.tile([128, 128], BF16)
            nc.sync.dma_start(triT_t[:], triT_d[:])
            nc.sync.dma_start(i128_t[:], i128_d[:])
            wo_t = wts.tile([128, 8, GROUP_DIMS], BF16)
            bo_t = cst.tile([128, GROUP_DIMS], F32)
            nc.sync.dma_start(wo_t[:], wo_d[:])
            nc.sync.dma_start(bo_t[:], bo_d[:])

            ccin = dram.tile([NQB, 4, 128, QB], BF16)
            ccout = dram.tile([NQB, 8, 128, QB], BF16)
            cc3 = dram.tile([4, 128, QB], BF16)   # qb=3 split B-half gathers

            # concat-dim chunk ck of cf <- ccout slot: A-half slots 0-3 hold
            # concat chunks [0,1,4,5] (g0.h0,g0.h1,g1.h0,g1.h1); B-half
            # slots 4-7 hold [2,3,6,7]
            CC_SLOT = {0: 0, 1: 1, 4: 2, 5: 3, 2: 4, 3: 5, 6: 6, 7: 7}
            CK_ORDER = [0, 1, 4, 5, 2, 3, 6, 7]  # A-half chunks first

            def phase3_gen(qb):
                """out-proj for chunk qb from the AllGathered full concat;
                A-half chunks accumulate while the B-half exchange lands."""
                cf = cfp.tile([128, 8, QB], BF16, tag="cf")
                for ck in CK_ORDER:
                    nc.sync.dma_start(cf[:, ck, :], ccout[qb, CC_SLOT[ck]])
                yield
                for st in range(4):
                    sq = 4 * qb + st
                    pso = ps.tile([128, GROUP_DIMS], F32, tag="ps")
                    for j, ck in enumerate(CK_ORDER):
                        nc.tensor.matmul(pso[:],
                                         cf[:, ck, st * 128:(st + 1) * 128],
                                         wo_t[:, ck, :],
                                         start=(j == 0), stop=(j == 7))
                        if j % 2 == 1:
                            yield
                    ot = outp.tile([128, GROUP_DIMS], BF16, tag="out")
                    nc.vector.tensor_add(ot[:], pso[:], bo_t[:])
                    nc.sync.dma_start(y_d[sq * 128:(sq + 1) * 128, :], ot[:])
                    yield

            work = []

            def pump(n=1):
                for _ in range(n):
                    advanced = False
                    while work:
                        try:
                            next(work[0])
                            advanced = True
                            break
                        except StopIteration:
                            work.pop(0)
                    if not advanced:
                        return

            p3_pending = None
            for qb in range(NQB):
                if qb + 2 < NSB:
                    issue_xp(qb + 2)
                if qb < NQB - 1:
                    work.append(phase1_gen(qb + 1))
                for hp in range(4):
                    # phase-3 of the previous chunk joins the fill queue
                    # only once its AllGather has certainly completed
                    if p3_pending is not None and hp >= (1 if qb == 3 else 2):
                        work.append(p3_pending)
                        p3_pending = None
                    if qb == 3 and hp == 3:
                        # pre-issue the tail's early gather reads (A-half +
                        # first split B pair are long since landed) so only
                        # cf3[3]/cf3[7] trail the final AllGather
                        cf3 = cfp.tile([128, 8, QB], BF16, tag="cf")
                        for ck in [0, 1, 4, 5]:
                            nc.sync.dma_start(cf3[:, ck, :],
                                              ccout[3, CC_SLOT[ck]])
                        nc.sync.dma_start(cf3[:, 2, :], cc3[0])
                        nc.sync.dma_start(cf3[:, 6, :], cc3[1])
                    oab = ps_acc.tile([128, 2, QB], F32, tag="oab")
                    nkt = 4 * qb + 4
                    for kt in range(nkt):
                        d = kt - 4 * qb
                        n0 = max(0, 128 * d)
                        n1 = QB
                        qs0 = qb * QB + n0
                        qs1 = (qb + 1) * QB
                        masked = d >= 0
                        s2 = ps_s.tile([128, 2, QB], F32, tag="s")
                        nc.tensor.matmul(s2[:, 0, n0:n1],
                                         kT[0:64, hp, kt * 128:(kt + 1) * 128],
                                         qT[0:64, hp, qs0:qs1],
                                         start=True, stop=not masked,
                                         tile_position=(0, 0))
                        nc.tensor.matmul(s2[:, 1, n0:n1],
                                         kT[64:128, hp, kt * 128:(kt + 1) * 128],
                                         qT[64:128, hp, qs0:qs1],
                                         start=True, stop=not masked,
                                         tile_position=(64, 0))
                        if masked:
                            # additive causal mask on the diagonal block via
                            # PE: s2[:, h, n0:n0+128] += triT.T @ I = tri
                            nc.tensor.matmul(s2[:, 0, n0:n0 + 128],
                                             triT_t[:], i128_t[:],
                                             start=False, stop=True)
                            nc.tensor.matmul(s2[:, 1, n0:n0 + 128],
                                             triT_t[:], i128_t[:],
                                             start=False, stop=True)
                        e2 = att.tile([128, 2, QB], BF16, tag="exp")
                        nc.scalar.activation(e2[:, :, n0:n1], s2[:, :, n0:n1],
                                             AFT.Exp, scale=0.125)
                        first = kt == 0
                        last = kt == nkt - 1
                        # fused attn@v + rowsum: stationary [vA|1] / [1|vB]
                        nc.tensor.matmul(oab[:, 0, n0:n1],
                                         v_sb[:, kt, hp, 0:128],
                                         e2[:, 0, n0:n1], start=first, stop=last)
                        nc.tensor.matmul(oab[:, 1, n0:n1],
                                         v_sb[:, kt, hp, 128:256],
                                         e2[:, 1, n0:n1], start=first, stop=last)
                        pump(2 if qb < 2 else 1)
                    # normalize: one full-height copy out of PSUM, one joint
                    # reciprocal over both heads' rowsum rows, two muls.
                    # (DVE PSUM reads must be full-height base-0; SBUF
                    # shift-down reads are fine, shift-up copies are not --
                    # head B's rows 64:128 placement goes via DMA.)
                    st2 = rcp.tile([128, 2, QB], F32, tag="st2")
                    nc.vector.tensor_copy(st2[:], oab[:])
                    rt2 = rcp.tile([64, 2, QB], F32, tag="rt2")
                    nc.vector.tensor_copy(rt2[:], st2[64:128, :, :])
                    rc2 = rcp.tile([64, 2, QB], F32, tag="rc2")
                    nc.vector.reciprocal_approx_fast(rc2[:], rt2[:])
                    # stage the chunk straight into the exchange buffer:
                    # head A rows 0:64, head B rows 64:128 of ccin[qb, hp]
                    oan = rcp.tile([64, QB], BF16, tag="oan")
                    nc.vector.tensor_mul(oan[:], st2[0:64, 0, :], rc2[:, 0, :])
                    obn = rcp.tile([64, QB], BF16, tag="obn")
                    nc.vector.tensor_mul(obn[:], st2[0:64, 1, :], rc2[:, 1, :])
                    nc.sync.dma_start(ccin[qb, hp, 0:64, :], oan[:])
                    nc.sync.dma_start(ccin[qb, hp, 64:128, :], obn[:])
                    if qb < 3:
                        if hp % 2 == 1:
                            half = hp // 2
                            nc.gpsimd.collective_compute(
                                "AllGather",
                                mybir.AluOpType.bypass,
                                replica_groups=[[0, 1], [2, 3], [4, 5], [6, 7]],
                                ins=[ccin[qb, 2 * half:2 * half + 2].opt()],
                                outs=[ccout[qb, 4 * half:4 * half + 4].opt()],
                            )
                    else:
                        # qb=3: A-half as one AG after hp1; B-half split per
                        # chunk so the tail only waits on the last small AG
                        if hp == 1:
                            nc.gpsimd.collective_compute(
                                "AllGather",
                                mybir.AluOpType.bypass,
                                replica_groups=[[0, 1], [2, 3], [4, 5], [6, 7]],
                                ins=[ccin[3, 0:2].opt()],
                                outs=[ccout[3, 0:4].opt()],
                            )
                        elif hp == 2:
                            nc.gpsimd.collective_compute(
                                "AllGather",
                                mybir.AluOpType.bypass,
                                replica_groups=[[0, 1], [2, 3], [4, 5], [6, 7]],
                                ins=[ccin[3, 2:3].opt()],
                                outs=[cc3[0:2].opt()],
                            )
                        elif hp == 3:
                            nc.gpsimd.collective_compute(
                                "AllGather",
                                mybir.AluOpType.bypass,
                                replica_groups=[[0, 1], [2, 3], [4, 5], [6, 7]],
                                ins=[ccin[3, 3:4].opt()],
                                outs=[cc3[2:4].opt()],
                            )
                while work:
                    pump()
                if qb < 3:
                    p3_pending = phase3_gen(qb)

            # ---- tail: phase 3 of the last chunk. All chunks except
            # [3, 7] are available before/at the last AG; accumulate them
            # into four live psums, then finish with [3, 7] + bias. ----
            # cc3 slots: [g0.hp2, g1.hp2, g0.hp3, g1.hp3]
            #          = concat chunks [2, 6, 3, 7]
            cf = cf3
            nc.sync.dma_start(cf[:, 3, :], cc3[2])
            nc.sync.dma_start(cf[:, 7, :], cc3[3])
            # warm-keeper: the PE would otherwise idle ~12us in the final
            # AllGather's shadow and HAM re-throttles it to 1.2GHz right
            # before the last 8 matmuls. Burn the idle window with dummy
            # matmuls (never read) so the tail runs at full clock.
            # st0/st1 accumulate in the pump ring (free since phase 3 of
            # qb=2 retired) so their matmuls start the moment the last
            # attn@v drains; st2/st3 take the score ring once the final
            # exp's read completes.
            so_a = ps.tile([128, GROUP_DIMS], F32, tag="ps")
            so_b = ps.tile([128, GROUP_DIMS], F32, tag="ps")
            so_c = ps_s.tile([128, 2, QB], F32, tag="s")
            psos = [so_a[:], so_b[:], so_c[:, 0, :], so_c[:, 1, :]]
            # A-half chunks for every st first, then the AG-B1 pair, then
            # the last AG-B2 pair -- keeps ready matmuls ahead of any
            # still-in-flight gather in the in-order PE queue.
            for st in range(4):
                for j, ck in enumerate([0, 1, 4, 5]):
                    nc.tensor.matmul(psos[st],
                                     cf[:, ck, st * 128:(st + 1) * 128],
                                     wo_t[:, ck, :],
                                     start=(j == 0), stop=False)
            for st in range(4):
                for ck in [2, 6]:
                    nc.tensor.matmul(psos[st],
                                     cf[:, ck, st * 128:(st + 1) * 128],
                                     wo_t[:, ck, :],
                                     start=False, stop=False)
            # keep the PE clocked through the final AllGather's shadow so
            # the gated matmuls below run warm (HAM re-throttles after
            # ~3.4us idle); the second score-ring slot is free by now
            warm = ps_s.tile([128, 2, QB], F32, tag="s")
            for _ in range(22):
                nc.tensor.matmul(warm[:, 0, :], wo_t[:, 0, 0:128], wo_t[:, 1, :],
                                 start=True, stop=True)
            for st in range(4):
                for j, ck in enumerate([3, 7]):
                    nc.tensor.matmul(psos[st],
                                     cf[:, ck, st * 128:(st + 1) * 128],
                                     wo_t[:, ck, :],
                                     start=False, stop=(j == 1))
                sq = 12 + st
                ot = outp.tile([128, GROUP_DIMS], BF16, tag="out")
                nc.vector.tensor_add(ot[:], psos[st], bo_t[:])
                nc.sync.dma_start(y_d[sq * 128:(sq + 1) * 128, :], ot[:])

    nc.compile()
    return nc


def _get_program():
    global _PROG
    if _PROG is None:
        _PROG = _build_program()
    return _PROG


def kernel(x, pos_emb, Wq, bq, Wk, bk, Wv, bv, Wp, bp, Wo, bo):
    x = np.asarray(x, dtype=np.float32)
    pos_emb = np.asarray(pos_emb, dtype=np.float32)
    Wq, bq = np.asarray(Wq, np.float32), np.asarray(bq, np.float32)
    Wk, bk = np.asarray(Wk, np.float32), np.asarray(bk, np.float32)
    Wv, bv = np.asarray(Wv, np.float32), np.asarray(bv, np.float32)
    Wp, bp = np.asarray(Wp, np.float32), np.asarray(bp, np.float32)
    Wo, bo = np.asarray(Wo, np.float32), np.asarray(bo, np.float32)

    nc = _get_program()

    # host-side pos projection, shared across cores up to the head slice
    p_full = pos_emb @ Wp.T + bp                                # [S, D]
    triT = np.where(np.arange(128)[:, None] >= np.arange(128)[None, :],
                    np.float32(0.0), np.float32(MASK_NEG))
    i128 = np.eye(128, dtype=np.float32)

    in_maps = []
    for c in range(N_CORES):
        b, g = divmod(c, 2)
        sl = slice(g * GROUP_DIMS, (g + 1) * GROUP_DIMS)
        xT = np.ascontiguousarray(x[b].T)                       # [D, S]
        xpa = xT.reshape(8, 128, NSB, SB).transpose(2, 0, 1, 3)  # [sb,i,p,u]
        wkT = Wk[sl].T                                          # [D, 512]
        wk2 = wkT.reshape(8, 128, 4, 128).transpose(2, 1, 0, 3)  # [dt,p,i,c]
        wv2 = Wv[sl].T.reshape(8, 128, GROUP_DIMS).transpose(1, 0, 2)
        wq2 = Wq[sl].T.reshape(8, 128, GROUP_DIMS).transpose(1, 0, 2)
        pq = p_full[:, sl] + bq[sl]                             # [S, 512]
        pq2 = pq.T.reshape(4, 128, NSB, SB).transpose(2, 1, 0, 3)  # [sb,p,dt,u]
        woh = Wo[sl, :].T                                       # [D, 512]
        wo2 = woh.reshape(8, 128, GROUP_DIMS).transpose(1, 0, 2)
        bk2 = bk[sl].reshape(4, 128).T
        bo_eff = bo[sl] + bv @ woh                              # [512]
        bo_bc = np.broadcast_to(bo_eff, (128, GROUP_DIMS))
        in_maps.append({
            "xpa": np.ascontiguousarray(xpa, dtype=ml_dtypes.bfloat16),
            "wk": np.ascontiguousarray(wk2, dtype=ml_dtypes.bfloat16),
            "wv": np.ascontiguousarray(wv2, dtype=ml_dtypes.bfloat16),
            "wq": np.ascontiguousarray(wq2, dtype=ml_dtypes.bfloat16),
            "pq": np.ascontiguousarray(pq2, dtype=ml_dtypes.bfloat16),
            "wo": np.ascontiguousarray(wo2, dtype=ml_dtypes.bfloat16),
            "bk": np.ascontiguousarray(bk2, dtype=np.float32),
            "bo_bc": np.ascontiguousarray(bo_bc, dtype=np.float32),
            "triT": np.ascontiguousarray(triT, dtype=ml_dtypes.bfloat16),
            "i128": np.ascontiguousarray(i128, dtype=ml_dtypes.bfloat16),
        })

    global _last_in_maps
    _last_in_maps = in_maps

    res = run_bass_kernel_spmd(nc, in_maps, list(range(N_CORES)))
    out = np.stack(
        [np.concatenate([res.results[2 * b]["y"], res.results[2 * b + 1]["y"]],
                        axis=1) for b in range(B)], axis=0)
    return out.astype(np.float32)


# revision 36
# speedup vs baseline: 1.0261x; 1.0261x over previous
"""Causal relative multi-head attention (prefill) on 8 Trainium2 NeuronCores.

Reference computation (fp32):
    q = x @ Wq.T + bq ; k = x @ Wk.T + bk ; v = x @ Wv.T + bv      [B,S,D]
    p = pos @ Wp.T + bp                                            [S,D]
    scores = causal((q+p) @ k.T / sqrt(dk)) ; attn = softmax(scores)
    out = (attn @ v) @ Wo.T + bo                                   [B,S,D]
with B=4, S=2048, D=1024, H=16, dk=64.

Sharding: batch x head-group. Core c handles batch b=c//2 and head group
g=c%2 (8 heads = 512 of the 1024 qkv/concat dims). After attention, the
pair {2b, 2b+1} exchanges bf16 attention outputs per 512-query chunk via
a small AllGather; each core then computes out-proj columns 512g:512(g+1)
over the full 1024-dim concat and writes its y half. Host stitches halves.

Key structure (v1):
  - pos projection p = pos@Wp.T + bp + bq is computed on HOST and shipped
    as pq [128,4,S] bf16; the device q-proj contracts only over x (8 steps)
    and adds pq with one DVE tensor_tensor per (dt, block).
  - 1/sqrt(dk) is folded into the exp activation's free scale operand
    (exp(0.125*s)), so no score scaling op exists anywhere.
  - The causal mask is applied on the PE: a tri[128,128] bf16 stationary
    (-240000 above the diagonal) matmul'd with an identity moving operand
    accumulates into the diagonal score block, so DVE is off the
    score->exp critical path.
  - Normalize: attn@v+rowsum accumulate into one [128,2,QB] psum pair
    (head A/B); one full-height copy to SBUF, one joint reciprocal over
    both heads' rowsum rows, two muls.
  - Tail: qb=3's B-half exchange is split per head-pair chunk (two
    1-chunk AllGathers) and phase 3 of the last chunk accumulates all
    early-available concat chunks first, so only the last small AG plus
    8 matmuls + bias are exposed at the end.
  - Startup: block-0 x and the k-weights are DMA'd in consumption-order
    chunks so the first matmul issues after ~400KB, not ~6MB.
"""

import numpy as np
import ml_dtypes

import concourse.bacc as bacc
import concourse.mybir as mybir
import concourse.tile as tile
from concourse.bass_utils import run_bass_kernel_spmd

F32 = mybir.dt.float32
BF16 = mybir.dt.bfloat16
F8 = mybir.dt.float8e4
DR = mybir.MatmulPerfMode.DoubleRow
AFT = mybir.ActivationFunctionType
ALU = mybir.AluOpType

B, S, D = 4, 2048, 1024
H, DK = 16, 64
N_CORES = 8
GROUP_DIMS = 512              # qkv dims per head group (8 heads x 64)
SB = 512                      # phase-1 seq block
NSB = S // SB                 # 4
QB = 512                      # phase-2 query block / output chunk
NQB = S // QB                 # 4
MASK_NEG = -240000.0          # pre-divided by the 0.125 exp scale

_PROG = None
_last_in_maps = None


def _build_program():
    nc = bacc.Bacc("TRN2", target_bir_lowering=False, debug=False,
                   num_devices=N_CORES)

    xpa_d = nc.dram_tensor("xpa", [NSB, 8, 128, SB], BF16, kind="ExternalInput")
    wk_d = nc.dram_tensor("wk", [4, 128, 8, 128], BF16, kind="ExternalInput")
    wv_d = nc.dram_tensor("wv", [128, 8, GROUP_DIMS], BF16, kind="ExternalInput")
    wq_d = nc.dram_tensor("wq", [128, 8, GROUP_DIMS], BF16, kind="ExternalInput")
    pq_d = nc.dram_tensor("pq", [NSB, 128, 4, SB], BF16, kind="ExternalInput")
    wo_d = nc.dram_tensor("wo", [128, 8, GROUP_DIMS], BF16, kind="ExternalInput")
    bk_d = nc.dram_tensor("bk", [128, 4], F32, kind="ExternalInput")
    bo_d = nc.dram_tensor("bo_bc", [128, GROUP_DIMS], F32, kind="ExternalInput")
    triT_d = nc.dram_tensor("triT", [128, 128], BF16, kind="ExternalInput")
    i128_d = nc.dram_tensor("i128", [128, 128], BF16, kind="ExternalInput")
    y_d = nc.dram_tensor("y", [S, GROUP_DIMS], BF16, kind="ExternalOutput")

    with tile.TileContext(nc) as tc:
        with (
            tc.tile_pool(name="wts", bufs=1) as wts,
            tc.tile_pool(name="xinA", bufs=3) as xinA,
            tc.tile_pool(name="big", bufs=1) as big,
            tc.tile_pool(name="att", bufs=3) as att,
            tc.tile_pool(name="rcp", bufs=2) as rcp,
            tc.tile_pool(name="cfp", bufs=2) as cfp,
            tc.tile_pool(name="outp", bufs=4) as outp,
            tc.tile_pool(name="cst", bufs=1) as cst,
            tc.tile_pool(name="ps", bufs=2, space="PSUM") as ps,
            tc.tile_pool(name="ps_s", bufs=2, space="PSUM") as ps_s,
            tc.tile_pool(name="ps_acc", bufs=1, space="PSUM") as ps_acc,
            tc.tile_pool(name="dram", bufs=1, space="DRAM") as dram,
        ):
            # ---- early DMAs in consumption order: k weights (dt chunks)
            # interleaved with block-0 x slices, then v/q weights + pq ----
            bk_t = cst.tile([128, 4], F32)
            nc.sync.dma_start(bk_t[:], bk_d[:])
            wk_t = wts.tile([128, 4, 8, 128], BF16)
            wv_t = wts.tile([128, 8, GROUP_DIMS], BF16)
            wq_t = wts.tile([128, 8, GROUP_DIMS], BF16)
            pq_t = wts.tile([128, 4, S], BF16)
            xp_tiles = {}

            def issue_xp(sb):
                ta = xinA.tile([128, 8, SB], BF16, tag="xpa")
                for i in range(8):
                    nc.sync.dma_start(ta[:, i, :], xpa_d[sb, i])
                nc.sync.dma_start(pq_t[:, :, sb * SB:(sb + 1) * SB], pq_d[sb])
                xp_tiles[sb] = ta

            ta0 = xinA.tile([128, 8, SB], BF16, tag="xpa")
            nc.scalar.dma_start(wk_t[:, 0], wk_d[0])
            nc.scalar.dma_start(ta0[:, 0, :], xpa_d[0, 0])
            nc.scalar.dma_start(ta0[:, 1, :], xpa_d[0, 1])
            nc.scalar.dma_start(wk_t[:, 1], wk_d[1])
            nc.scalar.dma_start(ta0[:, 2, :], xpa_d[0, 2])
            nc.scalar.dma_start(ta0[:, 3, :], xpa_d[0, 3])
            nc.sync.dma_start(wk_t[:, 2], wk_d[2])
            nc.sync.dma_start(ta0[:, 4, :], xpa_d[0, 4])
            nc.sync.dma_start(ta0[:, 5, :], xpa_d[0, 5])
            nc.sync.dma_start(wk_t[:, 3], wk_d[3])
            nc.sync.dma_start(ta0[:, 6, :], xpa_d[0, 6])
            nc.sync.dma_start(ta0[:, 7, :], xpa_d[0, 7])
            xp_tiles[0] = ta0
            nc.sync.dma_start(wv_t[:], wv_d[:])
            nc.sync.dma_start(pq_t[:, :, 0:SB], pq_d[0])
            nc.sync.dma_start(wq_t[:], wq_d[:])

            qT = big.tile([128, 4, S], BF16)      # q+p (unscaled), dims x seq
            kT = big.tile([128, 4, S], BF16)
            v_sb = big.tile([128, S // 128, 4, 256], BF16)  # [vA|1|vB|1]
            nc.gpsimd.memset(v_sb[:, :, :, 64:128], 1.0)
            nc.gpsimd.memset(v_sb[:, :, :, 192:256], 1.0)

            def phase1_gen(sb):
                """k -> v -> q projections for seq block sb; yields after
                small instruction bundles so it can be woven into phase 2."""
                xpa = xp_tiles.pop(sb)
                sbsl = slice(sb * SB, (sb + 1) * SB)
                for dt in range(4):
                    psk = ps.tile([128, GROUP_DIMS], F32, tag="ps")
                    for i in range(8):
                        nc.tensor.matmul(psk[:, :SB],
                                         wk_t[:, dt, i, :],
                                         xpa[:, i, :],
                                         start=(i == 0), stop=(i == 7))
                        if i % 2 == 1:
                            yield
                    nc.vector.tensor_scalar_add(
                        kT[:, dt, sbsl], psk[:, :SB], bk_t[:, dt:dt + 1])
                    yield
                for st in range(4):
                    psv = ps.tile([128, GROUP_DIMS], F32, tag="ps")
                    for i in range(8):
                        nc.tensor.matmul(psv[:],
                                         xpa[:, i, st * 128:(st + 1) * 128],
                                         wv_t[:, i, :],
                                         start=(i == 0), stop=(i == 7))
                        if i % 2 == 1:
                            yield
                    pv = psv[:].rearrange("p (a c) -> p a c", a=4)
                    t = sb * 4 + st
                    nc.vector.tensor_copy(v_sb[:, t, :, 0:64], pv[:, :, 0:64])
                    nc.vector.tensor_copy(v_sb[:, t, :, 128:192], pv[:, :, 64:128])
                    yield
                for dt in range(4):
                    psq = ps.tile([128, GROUP_DIMS], F32, tag="ps")
                    for i in range(8):
                        nc.tensor.matmul(psq[:, :SB],
                                         wq_t[:, i, dt * 128:(dt + 1) * 128],
                                         xpa[:, i, :],
                                         start=(i == 0), stop=(i == 7))
                        if i % 2 == 1:
                            yield
                    # cast PSUM fp32 -> bf16 first so the add is a clean
                    # all-bf16 tensor_tensor (2x DVE mode, no mixed dtypes)
                    qtmp = rcp.tile([128, SB], BF16, tag="qtmp")
                    nc.vector.tensor_copy(qtmp[:], psq[:, :SB])
                    yield
                    nc.vector.tensor_add(
                        qT[:, dt, sbsl], qtmp[:], pq_t[:, dt, sbsl])
                    yield

            for _ in phase1_gen(0):
                pass
            issue_xp(1)

            triT_t = cst.tile([128, 128], BF16)
            i128_t = cst.tile([128, 128], BF16)
            nc.sync.dma_start(triT_t[:], triT_d[:])
            nc.sync.dma_start(i128_t[:], i128_d[:])
            wo_t = wts.tile([128, 8, GROUP_DIMS], BF16)
            bo_t = cst.tile([128, GROUP_DIMS], F32)
            nc.sync.dma_start(wo_t[:], wo_d[:])
            nc.sync.dma_start(bo_t[:], bo_d[:])

            ccin = dram.tile([NQB, 4, 128, QB], BF16)
            ccout = dram.tile([NQB, 8, 128, QB], BF16)
            cc3 = dram.tile([4, 128, QB], BF16)   # qb=3 split B-half gathers

            # concat-dim chunk ck of cf <- ccout slot: A-half slots 0-3 hold
            # concat chunks [0,1,4,5] (g0.h0,g0.h1,g1.h0,g1.h1); B-half
            # slots 4-7 hold [2,3,6,7]
            CC_SLOT = {0: 0, 1: 1, 4: 2, 5: 3, 2: 4, 3: 5, 6: 6, 7: 7}
            CK_ORDER = [0, 1, 4, 5, 2, 3, 6, 7]  # A-half chunks first

            def phase3_gen(qb):
                """out-proj for chunk qb from the AllGathered full concat;
                A-half chunks accumulate while the B-half exchange lands."""
                cf = cfp.tile([128, 8, QB], BF16, tag="cf")
                for ck in CK_ORDER:
                    nc.sync.dma_start(cf[:, ck, :], ccout[qb, CC_SLOT[ck]])
                yield
                for st in range(4):
                    sq = 4 * qb + st
                    pso = ps.tile([128, GROUP_DIMS], F32, tag="ps")
                    for j, ck in enumerate(CK_ORDER):
                        nc.tensor.matmul(pso[:],
                                         cf[:, ck, st * 128:(st + 1) * 128],
                                         wo_t[:, ck, :],
                                         start=(j == 0), stop=(j == 7))
                        if j % 2 == 1:
                            yield
                    ot = outp.tile([128, GROUP_DIMS], BF16, tag="out")
                    nc.vector.tensor_add(ot[:], pso[:], bo_t[:])
                    nc.sync.dma_start(y_d[sq * 128:(sq + 1) * 128, :], ot[:])
                    yield

            work = []

            def pump(n=1):
                for _ in range(n):
                    advanced = False
                    while work:
                        try:
                            next(work[0])
                            advanced = True
                            break
                        except StopIteration:
                            work.pop(0)
                    if not advanced:
                        return

            p3_pending = None
            for qb in range(NQB):
                if qb + 2 < NSB:
                    issue_xp(qb + 2)
                if qb < NQB - 1:
                    work.append(phase1_gen(qb + 1))
                for hp in range(4):
                    # phase-3 of the previous chunk joins the fill queue
                    # only once its AllGather has certainly completed
                    if p3_pending is not None and hp >= (1 if qb == 3 else 2):
                        work.append(p3_pending)
                        p3_pending = None
                    if qb == 3 and hp == 3:
                        # pre-issue the tail's early gather reads (A-half +
                        # first split B pair are long since landed) so only
                        # cf3[3]/cf3[7] trail the final AllGather
                        cf3 = cfp.tile([128, 8, QB], BF16, tag="cf")
                        for ck in [0, 1, 4, 5]:
                            nc.sync.dma_start(cf3[:, ck, :],
                                              ccout[3, CC_SLOT[ck]])
                        nc.sync.dma_start(cf3[:, 2, :], cc3[0])
                        nc.sync.dma_start(cf3[:, 6, :], cc3[1])
                    oab = ps_acc.tile([128, 2, QB], F32, tag="oab")
                    nkt = 4 * qb + 4
                    for kt in range(nkt):
                        d = kt - 4 * qb
                        n0 = max(0, 128 * d)
                        n1 = QB
                        qs0 = qb * QB + n0
                        qs1 = (qb + 1) * QB
                        masked = d >= 0
                        s2 = ps_s.tile([128, 2, QB], F32, tag="s")
                        nc.tensor.matmul(s2[:, 0, n0:n1],
                                         kT[0:64, hp, kt * 128:(kt + 1) * 128],
                                         qT[0:64, hp, qs0:qs1],
                                         start=True, stop=not masked,
                                         tile_position=(0, 0))
                        nc.tensor.matmul(s2[:, 1, n0:n1],
                                         kT[64:128, hp, kt * 128:(kt + 1) * 128],
                                         qT[64:128, hp, qs0:qs1],
                                         start=True, stop=not masked,
                                         tile_position=(64, 0))
                        if masked:
                            # additive causal mask on the diagonal block via
                            # PE: s2[:, h, n0:n0+128] += triT.T @ I = tri
                            nc.tensor.matmul(s2[:, 0, n0:n0 + 128],
                                             triT_t[:], i128_t[:],
                                             start=False, stop=True)
                            nc.tensor.matmul(s2[:, 1, n0:n0 + 128],
                                             triT_t[:], i128_t[:],
                                             start=False, stop=True)
                        e2 = att.tile([128, 2, QB], BF16, tag="exp")
                        nc.scalar.activation(e2[:, :, n0:n1], s2[:, :, n0:n1],
                                             AFT.Exp, scale=0.125)
                        first = kt == 0
                        last = kt == nkt - 1
                        # fused attn@v + rowsum: stationary [vA|1] / [1|vB]
                        nc.tensor.matmul(oab[:, 0, n0:n1],
                                         v_sb[:, kt, hp, 0:128],
                                         e2[:, 0, n0:n1], start=first, stop=last)
                        nc.tensor.matmul(oab[:, 1, n0:n1],
                                         v_sb[:, kt, hp, 128:256],
                                         e2[:, 1, n0:n1], start=first, stop=last)
                        pump(2 if qb < 2 else 1)
                    # normalize: one full-height copy out of PSUM, one joint
                    # reciprocal over both heads' rowsum rows, two muls.
                    # (DVE PSUM reads must be full-height base-0; SBUF
                    # shift-down reads are fine, shift-up copies are not --
                    # head B's rows 64:128 placement goes via DMA.)
                    st2 = rcp.tile([128, 2, QB], F32, tag="st2")
                    nc.vector.tensor_copy(st2[:], oab[:])
                    rt2 = rcp.tile([64, 2, QB], F32, tag="rt2")
                    nc.vector.tensor_copy(rt2[:], st2[64:128, :, :])
                    rc2 = rcp.tile([64, 2, QB], F32, tag="rc2")
                    nc.vector.reciprocal_approx_fast(rc2[:], rt2[:])
                    # stage the chunk straight into the exchange buffer:
                    # head A rows 0:64, head B rows 64:128 of ccin[qb, hp]
                    oan = rcp.tile([64, QB], BF16, tag="oan")
                    nc.vector.tensor_mul(oan[:], st2[0:64, 0, :], rc2[:, 0, :])
                    obn = rcp.tile([64, QB], BF16, tag="obn")
                    nc.vector.tensor_mul(obn[:], st2[0:64, 1, :], rc2[:, 1, :])
                    nc.sync.dma_start(ccin[qb, hp, 0:64, :], oan[:])
                    nc.sync.dma_start(ccin[qb, hp, 64:128, :], obn[:])
                    if qb < 3:
                        if hp % 2 == 1:
                            half = hp // 2
                            nc.gpsimd.collective_compute(
                                "AllGather",
                                mybir.AluOpType.bypass,
                                replica_groups=[[0, 1], [2, 3], [4, 5], [6, 7]],
                                ins=[ccin[qb, 2 * half:2 * half + 2].opt()],
                                outs=[ccout[qb, 4 * half:4 * half + 4].opt()],
                            )
                    else:
                        # qb=3: A-half as one AG after hp1; B-half split per
                        # chunk so the tail only waits on the last small AG
                        if hp == 1:
                            nc.gpsimd.collective_compute(
                                "AllGather",
                                mybir.AluOpType.bypass,
                                replica_groups=[[0, 1], [2, 3], [4, 5], [6, 7]],
                                ins=[ccin[3, 0:2].opt()],
                                outs=[ccout[3, 0:4].opt()],
                            )
                        elif hp == 2:
                            nc.gpsimd.collective_compute(
                                "AllGather",
                                mybir.AluOpType.bypass,
                                replica_groups=[[0, 1], [2, 3], [4, 5], [6, 7]],
                                ins=[ccin[3, 2:3].opt()],
                                outs=[cc3[0:2].opt()],
                            )
                        elif hp == 3:
                            nc.gpsimd.collective_compute(
                                "AllGather",
                                mybir.AluOpType.bypass,
                                replica_groups=[[0, 1], [2, 3], [4, 5], [6, 7]],
                                ins=[ccin[3, 3:4].opt()],
                                outs=[cc3[2:4].opt()],
                            )
                while work:
                    pump()
                if qb < 3:
                    p3_pending = phase3_gen(qb)

            # ---- tail: phase 3 of the last chunk. All chunks except
            # [3, 7] are available before/at the last AG; accumulate them
            # into four live psums, then finish with [3, 7] + bias. ----
            # cc3 slots: [g0.hp2, g1.hp2, g0.hp3, g1.hp3]
            #          = concat chunks [2, 6, 3, 7]
            cf = cf3
            nc.sync.dma_start(cf[:, 3, :], cc3[2])
            nc.sync.dma_start(cf[:, 7, :], cc3[3])
            # warm-keeper: the PE would otherwise idle ~12us in the final
            # AllGather's shadow and HAM re-throttles it to 1.2GHz right
            # before the last 8 matmuls. Burn the idle window with dummy
            # matmuls (never read) so the tail runs at full clock.
            # st0/st1 accumulate in the pump ring (free since phase 3 of
            # qb=2 retired) so their matmuls start the moment the last
            # attn@v drains; st2/st3 take the score ring once the final
            # exp's read completes.
            so_a = ps.tile([128, GROUP_DIMS], F32, tag="ps")
            so_b = ps.tile([128, GROUP_DIMS], F32, tag="ps")
            so_c = ps_s.tile([128, 2, QB], F32, tag="s")
            psos = [so_a[:], so_b[:], so_c[:, 0, :], so_c[:, 1, :]]
            # A-half chunks for every st first, then the AG-B1 pair, then
            # the last AG-B2 pair -- keeps ready matmuls ahead of any
            # still-in-flight gather in the in-order PE queue.
            for st in range(4):
                for j, ck in enumerate([0, 1, 4, 5]):
                    nc.tensor.matmul(psos[st],
                                     cf[:, ck, st * 128:(st + 1) * 128],
                                     wo_t[:, ck, :],
                                     start=(j == 0), stop=False)
            for st in range(4):
                for ck in [2, 6]:
                    nc.tensor.matmul(psos[st],
                                     cf[:, ck, st * 128:(st + 1) * 128],
                                     wo_t[:, ck, :],
                                     start=False, stop=False)
            for st in range(4):
                for j, ck in enumerate([3, 7]):
                    nc.tensor.matmul(psos[st],
                                     cf[:, ck, st * 128:(st + 1) * 128],
                                     wo_t[:, ck, :],
                                     start=False, stop=(j == 1))
                sq = 12 + st
                ot = outp.tile([128, GROUP_DIMS], BF16, tag="out")
                nc.vector.tensor_add(ot[:], psos[st], bo_t[:])
                nc.sync.dma_start(y_d[sq * 128:(sq + 1) * 128, :], ot[:])

    nc.compile()
    return nc


def _get_program():
    global _PROG
    if _PROG is None:
        _PROG = _build_program()
    return _PROG


def kernel(x, pos_emb, Wq, bq, Wk, bk, Wv, bv, Wp, bp, Wo, bo):
    x = np.asarray(x, dtype=np.float32)
    pos_emb = np.asarray(pos_emb, dtype=np.float32)
    Wq, bq = np.asarray(Wq, np.float32), np.asarray(bq, np.float32)
    Wk, bk = np.asarray(Wk, np.float32), np.asarray(bk, np.float32)
    Wv, bv = np.asarray(Wv, np.float32), np.asarray(bv, np.float32)
    Wp, bp = np.asarray(Wp, np.float32), np.asarray(bp, np.float32)
    Wo, bo = np.asarray(Wo, np.float32), np.asarray(bo, np.float32)

    nc = _get_program()

    # host-side pos projection, shared across cores up to the head slice
    p_full = pos_emb @ Wp.T + bp                                # [S, D]
    triT = np.where(np.arange(128)[:, None] >= np.arange(128)[None, :],
                    np.float32(0.0), np.float32(MASK_NEG))
    i128 = np.eye(128, dtype=np.float32)

    in_maps = []
    for c in range(N_CORES):
        b, g = divmod(c, 2)
        sl = slice(g * GROUP_DIMS, (g + 1) * GROUP_DIMS)
        xT = np.ascontiguousarray(x[b].T)                       # [D, S]
        xpa = xT.reshape(8, 128, NSB, SB).transpose(2, 0, 1, 3)  # [sb,i,p,u]
        wkT = Wk[sl].T                                          # [D, 512]
        wk2 = wkT.reshape(8, 128, 4, 128).transpose(2, 1, 0, 3)  # [dt,p,i,c]
        wv2 = Wv[sl].T.reshape(8, 128, GROUP_DIMS).transpose(1, 0, 2)
        wq2 = Wq[sl].T.reshape(8, 128, GROUP_DIMS).transpose(1, 0, 2)
        pq = p_full[:, sl] + bq[sl]                             # [S, 512]
        pq2 = pq.T.reshape(4, 128, NSB, SB).transpose(2, 1, 0, 3)  # [sb,p,dt,u]
        woh = Wo[sl, :].T                                       # [D, 512]
        wo2 = woh.reshape(8, 128, GROUP_DIMS).transpose(1, 0, 2)
        bk2 = bk[sl].reshape(4, 128).T
        bo_eff = bo[sl] + bv @ woh                              # [512]
        bo_bc = np.broadcast_to(bo_eff, (128, GROUP_DIMS))
        in_maps.append({
            "xpa": np.ascontiguousarray(xpa, dtype=ml_dtypes.bfloat16),
            "wk": np.ascontiguousarray(wk2, dtype=ml_dtypes.bfloat16),
            "wv": np.ascontiguousarray(wv2, dtype=ml_dtypes.bfloat16),
            "wq": np.ascontiguousarray(wq2, dtype=ml_dtypes.bfloat16),
            "pq": np.ascontiguousarray(pq2, dtype=ml_dtypes.bfloat16),
            "wo": np.ascontiguousarray(wo2, dtype=ml_dtypes.bfloat16),
            "bk": np.ascontiguousarray(bk2, dtype=np.float32),
            "bo_bc": np.ascontiguousarray(bo_bc, dtype=np.float32),
            "triT": np.ascontiguousarray(triT, dtype=ml_dtypes.bfloat16),
            "i128": np.ascontiguousarray(i128, dtype=ml_dtypes.bfloat16),
        })

    global _last_in_maps
    _last_in_maps = in_maps

    res = run_bass_kernel_spmd(nc, in_maps, list(range(N_CORES)))
    out = np.stack(
        [np.concatenate([res.results[2 * b]["y"], res.results[2 * b + 1]["y"]],
                        axis=1) for b in range(B)], axis=0)
    return out.astype(np.float32)
